# revision 10
# baseline (speedup 1.0000x reference)
"""Trainium2 Bass kernel for nn_Derivative_78898549227959 (gnn_message_passing).

Computes, for x = where(discrete_mask, (inputs > 0), inputs)  [straight-through
forward value], per-node tiny MLPs with adjacency-masked inputs:

    h1 = relu(einsum('bd,ndh->bnh', x, A[n,d]*W1[n,d,h]) + b1)
    h2 = relu(einsum('bnh,nhk->bnk', h1, W2) + b2)
    out[b,n] = einsum('bnk,nk->bn', h2, W3) + b3

Distribution: data-parallel over 8 NeuronCores — batch B=8192 sharded into
8 x 1024; weights/adjacency replicated (SPMD, same program each core).

Kernel layout strategy (per core, BS=1024):
 - x is transposed on-chip to xT [d, b] via PE transposes; preprocessing
   (straight-through binarization) runs in the transposed layout where
   discrete_mask is a per-partition scalar.
 - L1 is a dense GEMM: out[nh, b] = W1m[d, nh]^T @ xT[d, b] with the
   adjacency folded into the weights (W1m = AT * W1) and the contraction
   padded to K=130 = 65 + 65 where the last row is ones/b1 (exact bias fold).
 - L2 uses block-diagonal [128,128] lhsT tiles holding W2 of a node pair;
   b2 is applied as a per-partition bias in the ACT relu eviction.
 - L3 uses [128,128] lhsT tiles that are zero except two columns (W3 of the
   node pair), so all 64 pairs accumulate into a single PSUM bank giving
   outT[n, b] directly; b3 is folded into the eviction add.
 - outT is PE-transposed back to [b, n] and stored with one DMA.
 - Matmuls run in float32r (1 cycle/row vs 4 for float32). The BIR verifier
   requires every producer of an fp32r matmul operand to be a compute op
   with fp32r-rounded output, so all operand tiles are written by DVE/ACT
   ops with fp32r-tagged out APs (DMA-loaded data is staged raw and
   round-copied).
"""

import sys

sys.path.insert(0, "/opt/trn_rl_repo")

import numpy as np

import concourse.bacc as bacc
import concourse.mybir as mybir
from concourse.bass_utils import run_bass_kernel_spmd
from concourse.masks import make_identity
from concourse.tile import TileContext

B = 8192
D = 129
H = 64
N_CORES = 8
BS = B // N_CORES          # 1024 batch rows per core
NCH = 8                    # BS / 128 partition chunks
NPAIR = 64                 # node pairs (0..127); node 128 handled separately
F32 = mybir.dt.float32
F32R = mybir.dt.float32r
I32 = mybir.dt.int32
MM_DT = F32R

AF = mybir.ActivationFunctionType
OP = mybir.AluOpType


def _mm(nc, out, lhsT, rhs, **kw):
    nc.tensor.matmul(out, lhsT.bitcast(MM_DT), rhs.bitcast(MM_DT), **kw)


def build():
    nc = bacc.Bacc("TRN2", target_bir_lowering=False, debug=False,
                   num_devices=N_CORES)

    d_inputs = nc.dram_tensor("inputs", [BS, D], F32, kind="ExternalInput")
    d_adj = nc.dram_tensor("adjacency", [D, D], F32, kind="ExternalInput")
    d_w1 = nc.dram_tensor("W1", [D, D, H], F32, kind="ExternalInput")
    d_b1 = nc.dram_tensor("b1", [D, H], F32, kind="ExternalInput")
    d_w2 = nc.dram_tensor("W2", [D, H, H], F32, kind="ExternalInput")
    d_b2 = nc.dram_tensor("b2", [D, H], F32, kind="ExternalInput")
    d_w3 = nc.dram_tensor("W3", [D, H], F32, kind="ExternalInput")
    d_b3 = nc.dram_tensor("b3", [D], F32, kind="ExternalInput")
    d_dm = nc.dram_tensor("discrete_mask", [D], I32, kind="ExternalInput")
    d_out = nc.dram_tensor("out", [BS, D], F32, kind="ExternalOutput")

    with TileContext(nc) as tc:
        with tc.tile_pool(name="consts", bufs=1) as consts:
            identity = consts.tile([128, 128], F32)
            make_identity(nc, identity)

            # persistent tiles
            xta = consts.tile([65, BS], F32)     # xT rows d=0..64
            xtb = consts.tile([65, BS], F32)     # xT rows d=65..128, row64=ones
            w1a = consts.tile([65, D * H], F32)  # masked W1, d=0..64
            w1b = consts.tile([65, D * H], F32)  # masked W1 d=65..128 + b1 row
            w2blk = consts.tile([128, 65 * 128], F32)
            w3pack = consts.tile([128, NPAIR * 128], F32)
            w3tfull = consts.tile([128, D], F32)  # W3T twice (rows 0-63, 64-127)
            at_a = consts.tile([65, D], F32)
            at_b = consts.tile([64, D], F32)
            b2pack = consts.tile([128, 65], F32)
            b3sb = consts.tile([1, D], F32)
            mta = consts.tile([65, 1], F32)
            mtb = consts.tile([64, 1], F32)
            b3col = consts.tile([128, 1], F32)
            outT = consts.tile([128, BS], F32)
            outThi = consts.tile([1, BS], F32)
            outf = consts.tile([128, NCH * D], F32)

            nc.sync.dma_start(out=b3sb, in_=d_b3.ap()[None, :])

            # ---------- stage 0: loads + transposes + x preprocessing ----------
            with (
                tc.tile_pool(name="stage0", bufs=1) as st0,
                tc.tile_pool(name="psum_t", bufs=2, space="PSUM") as pst,
            ):
                def tr(dst, src, tagged=False):
                    p = src.shape[0]
                    f = src.shape[1]
                    t = pst.tile([128, 128], F32, tag="tr", name="trp")
                    nc.tensor.transpose(t[0:f, 0:p], src, identity[0:p, 0:p])
                    d = dst.bitcast(F32R) if tagged else dst
                    nc.vector.tensor_copy(d, t[0:f, 0:p])

                xin = st0.tile([128, NCH * D], F32)
                nc.sync.dma_start(
                    out=xin.rearrange("p (c d) -> p c d", c=NCH),
                    in_=d_inputs.ap().rearrange("(c p) d -> p c d", p=128),
                )
                a_sb = st0.tile([128, D], F32)
                a_hi = st0.tile([1, D], F32)
                nc.sync.dma_start(out=a_sb, in_=d_adj.ap()[0:128])
                nc.sync.dma_start(out=a_hi, in_=d_adj.ap()[128:129])
                # W3 loaded twice side by side so its transpose lands on both
                # partition halves (needed for DVE-only packing below)
                w3dbl = st0.tile([128, 128], F32)
                w3dblhi = st0.tile([1, 128], F32)
                nc.sync.dma_start(out=w3dbl[:, 0:64], in_=d_w3.ap()[0:128])
                nc.sync.dma_start(out=w3dbl[:, 64:128], in_=d_w3.ap()[0:128])
                nc.sync.dma_start(out=w3dblhi[:, 0:64], in_=d_w3.ap()[128:129])
                nc.sync.dma_start(out=w3dblhi[:, 64:128], in_=d_w3.ap()[128:129])
                b2sb = st0.tile([128, H], F32)
                b2hi = st0.tile([1, H], F32)
                nc.sync.dma_start(out=b2sb, in_=d_b2.ap()[0:128])
                nc.sync.dma_start(out=b2hi, in_=d_b2.ap()[128:129])
                dm_i = st0.tile([1, D], I32)
                nc.sync.dma_start(out=dm_i, in_=d_dm.ap()[None, :])
                dm_f = st0.tile([1, D], F32)
                nc.vector.tensor_copy(dm_f, dm_i)
                b2t = st0.tile([64, D], F32)

                xv = xin.rearrange("p (c d) -> p c d", c=NCH)
                for c in range(NCH):
                    tr(xta[:, c * 128:(c + 1) * 128], xv[:, c, 0:65], tagged=True)
                    tr(xtb[0:64, c * 128:(c + 1) * 128], xv[:, c, 65:129],
                       tagged=True)
                tr(at_a[:, 0:128], a_sb[:, 0:65])
                tr(at_a[:, 128:129], a_hi[:, 0:65])
                tr(at_b[:, 0:128], a_sb[:, 65:129])
                tr(at_b[:, 128:129], a_hi[:, 65:129])
                tr(w3tfull[:, 0:128], w3dbl, tagged=True)
                tr(w3tfull[:, 128:129], w3dblhi, tagged=True)
                tr(b2t[:, 0:128], b2sb)
                tr(b2t[:, 128:129], b2hi)
                tr(mta, dm_f[:, 0:65])
                tr(mtb, dm_f[:, 65:129])
                tr(b3col, b3sb[:, 0:128])

                # b2 packed per-pair bias columns (feeds ACT bias only)
                nc.gpsimd.memset(b2pack, 0.0)
                nc.sync.dma_start(out=b2pack[0:64], in_=b2t[:, 0:129:2])
                nc.sync.dma_start(out=b2pack[64:128, 0:64], in_=b2t[:, 1:129:2])

                # x = inputs + m * ((inputs > 0) - inputs), m per-partition
                ha = st0.tile([65, BS], F32)
                hb = st0.tile([64, BS], F32)
                nc.vector.tensor_single_scalar(ha, xta, 0.0, OP.is_gt)
                nc.vector.tensor_sub(ha, ha, xta)
                nc.vector.scalar_tensor_tensor(
                    xta.bitcast(F32R), ha, mta, xta, OP.mult, OP.add)
                nc.vector.tensor_single_scalar(hb, xtb[0:64], 0.0, OP.is_gt)
                nc.vector.tensor_sub(hb, hb, xtb[0:64])
                nc.vector.scalar_tensor_tensor(
                    xtb[0:64].bitcast(F32R), hb, mtb, xtb[0:64],
                    OP.mult, OP.add)
                ones_raw = st0.tile([1, BS], F32)
                nc.vector.memset(ones_raw, 1.0)
                nc.vector.tensor_copy(xtb[64:65, :].bitcast(F32R), ones_raw)

            # ---------- stage 1: W1 load + adjacency masking (chunked) --------
            with tc.tile_pool(name="w1stage", bufs=2) as w1s:
                w1t = d_w1.ap().transpose([1, 0, 2])  # [d, n, h]
                w1a3 = w1a.rearrange("p (n h) -> p n h", n=D)
                w1b3 = w1b[0:64].rearrange("p (n h) -> p n h", n=D)
                chunks = [(q * 16, min(16, D - q * 16)) for q in range(9)]
                for n0, cnt in chunks:
                    raw = w1s.tile([65, 16 * H], F32, tag="w1raw", name="w1raw")
                    r3 = raw.rearrange("p (n h) -> p n h", n=16)[:, 0:cnt, :]
                    nc.sync.dma_start(out=r3, in_=w1t[0:65, n0:n0 + cnt, :])
                    nc.vector.tensor_tensor(
                        w1a3[:, n0:n0 + cnt, :].bitcast(F32R), r3,
                        at_a[:, n0:n0 + cnt, None].broadcast_to([65, cnt, H]),
                        OP.mult)
                for n0, cnt in chunks:
                    raw = w1s.tile([65, 16 * H], F32, tag="w1raw", name="w1raw")
                    r3 = raw.rearrange("p (n h) -> p n h", n=16)[0:64, 0:cnt, :]
                    nc.sync.dma_start(out=r3, in_=w1t[65:129, n0:n0 + cnt, :])
                    nc.vector.tensor_tensor(
                        w1b3[:, n0:n0 + cnt, :].bitcast(F32R), r3,
                        at_b[:, n0:n0 + cnt, None].broadcast_to([64, cnt, H]),
                        OP.mult)
                # bias row of the K=130 contraction: b1 flattened, rounded
                b1raw = w1s.tile([1, D * H], F32, bufs=1)
                nc.sync.dma_start(
                    out=b1raw, in_=d_b1.ap().rearrange("n h -> (n h)")[None, :])
                nc.vector.tensor_copy(w1b[64:65, :].bitcast(F32R), b1raw)

            # ---------- stage 2: W2 block-diagonal build ----------------------
            with tc.tile_pool(name="w2stage", bufs=1) as w2s:
                w2raw = w2s.tile([128, 65 * 128], F32)
                nc.gpsimd.memset(w2raw, 0.0)
                nc.sync.dma_start(
                    out=w2raw[0:64].rearrange("p (j q) -> p j q", q=128)[:, :, 0:64],
                    in_=d_w2.ap()[0:129:2].transpose([1, 0, 2]),
                )
                nc.sync.dma_start(
                    out=w2raw[64:128].rearrange(
                        "p (j q) -> p j q", q=128)[:, 0:64, 64:128],
                    in_=d_w2.ap()[1:129:2].transpose([1, 0, 2]),
                )
                nc.vector.tensor_copy(w2blk.bitcast(F32R), w2raw)

            # ---------- W3 packed tiles (DVE-only writers) --------------------
            # tile j: col 2j = [W3[2j]; 0], col 2j+1 = [0; W3[2j+1]]
            # column of pair j's block for node 2j is 128*j + 2j = 130*j
            # zero-fill via self-compare of an initialized broadcast source
            # (memset can't emit f32r)
            zsrc = identity[:, 0:1].broadcast_to([128, NPAIR * 128])
            nc.vector.tensor_tensor(w3pack.bitcast(F32R), zsrc, zsrc, OP.is_lt)
            nc.vector.tensor_copy(
                w3pack[0:64, 0:8191:130].bitcast(F32R), w3tfull[0:64, 0:128:2])
            nc.vector.tensor_copy(
                w3pack[64:128, 1:8192:130].bitcast(F32R),
                w3tfull[64:128, 1:128:2])

            # ---------- main per-pair pipeline --------------------------------
            with (
                tc.tile_pool(name="ps1", bufs=3, space="PSUM") as ps1,
                tc.tile_pool(name="ps2", bufs=3, space="PSUM") as ps2,
                tc.tile_pool(name="ps3", bufs=1, space="PSUM") as ps3,
                tc.tile_pool(name="ps3h", bufs=1, space="PSUM") as ps3h,
                tc.tile_pool(name="work", bufs=3) as work,
            ):
                for bc in range(2):
                    bs = slice(bc * 512, (bc + 1) * 512)
                    psum3 = ps3.tile([128, 512], F32, tag="psum3", name="psum3")
                    psum3hi = ps3h.tile([1, 512], F32, tag="psum3hi",
                                        name="psum3hi")
                    for j in range(65):
                        m = 128 if j < 64 else 64
                        cs = slice(j * 128, j * 128 + m)
                        psum1 = ps1.tile([128, 512], F32, tag="psum1",
                                         name="psum1")
                        _mm(nc, psum1[0:m], w1a[:, cs], xta[:, bs],
                            start=True, stop=False)
                        _mm(nc, psum1[0:m], w1b[:, cs], xtb[:, bs],
                            start=False, stop=True)
                        h1 = work.tile([128, 512], F32, tag="h1", name="h1")
                        nc.vector.tensor_relu(h1[0:m].bitcast(F32R), psum1[0:m])

                        psum2 = ps2.tile([128, 512], F32, tag="psum2",
                                         name="psum2")
                        _mm(nc, psum2[0:m], w2blk[0:m, cs], h1[0:m],
                            start=True, stop=True)
                        h2 = work.tile([128, 512], F32, tag="h2", name="h2")
                        nc.scalar.activation(
                            h2[0:m].bitcast(F32R), psum2[0:m], AF.Relu,
                            bias=b2pack[0:m, j:j + 1])

                        if j < 64:
                            _mm(nc, psum3, w3pack[:, cs], h2,
                                start=(j == 0), stop=(j == 63))
                        else:
                            _mm(nc, psum3hi, w3tfull[0:64, 128:129], h2[0:64],
                                start=True, stop=True)

                    nc.vector.tensor_scalar_add(outT[:, bs], psum3, b3col)
                    nc.vector.tensor_scalar_add(
                        outThi[:, bs], psum3hi, b3sb[:, 128:129])

            # ---------- transpose back to [b, n] and store --------------------
            outfv = outf.rearrange("p (c d) -> p c d", c=NCH)
            with tc.tile_pool(name="psum_o", bufs=3, space="PSUM") as pso:
                for c in range(NCH):
                    t = pso.tile([128, 128], F32, tag="tro", name="tro")
                    nc.tensor.transpose(
                        t, outT[:, c * 128:(c + 1) * 128], identity)
                    nc.vector.tensor_copy(outfv[:, c, 0:128], t)
                    t2 = pso.tile([128, 1], F32, tag="tro2", name="tro2")
                    nc.tensor.transpose(
                        t2, outThi[:, c * 128:(c + 1) * 128],
                        identity[0:1, 0:1])
                    nc.vector.tensor_copy(outfv[:, c, 128:129], t2)

            nc.sync.dma_start(
                out=d_out.ap().rearrange("(c p) d -> p c d", p=128),
                in_=outfv,
            )

            nc._dbg = dict(xta=xta, xtb=xtb, w1a=w1a, w1b=w1b, at_a=at_a,
                           at_b=at_b, w2blk=w2blk, w3pack=w3pack,
                           b2pack=b2pack, outT=outT, outThi=outThi,
                           mta=mta, mtb=mtb, b3col=b3col)

    nc.compile()
    return nc


_NC_CACHE = None


def get_nc():
    global _NC_CACHE
    if _NC_CACHE is None:
        _NC_CACHE = build()
    return _NC_CACHE


def kernel(inputs, adjacency, W1, b1, W2, b2, W3, b3, discrete_mask,
           trace=False, **trace_kwargs):
    nc = get_nc()
    shared = {
        "adjacency": np.ascontiguousarray(adjacency, np.float32),
        "W1": np.ascontiguousarray(W1, np.float32),
        "b1": np.ascontiguousarray(b1, np.float32),
        "W2": np.ascontiguousarray(W2, np.float32),
        "b2": np.ascontiguousarray(b2, np.float32),
        "W3": np.ascontiguousarray(W3, np.float32),
        "b3": np.ascontiguousarray(b3, np.float32),
        "discrete_mask": np.ascontiguousarray(discrete_mask, np.int32),
    }
    inputs = np.ascontiguousarray(inputs, np.float32)
    in_maps = [
        {"inputs": inputs[i * BS:(i + 1) * BS], **shared}
        for i in range(N_CORES)
    ]
    res = run_bass_kernel_spmd(nc, in_maps, list(range(N_CORES)),
                               trace=trace, **trace_kwargs)
    out = np.concatenate([res.results[i]["out"] for i in range(N_CORES)], axis=0)
    if trace:
        kernel.last_results = res
    return out
